# revision 2
# baseline (speedup 1.0000x reference)
"""AttentionWithRoPE on 8 trn2 NeuronCores.

Sharding (tensor-parallel over heads x data-parallel over batch):
  core c -> batch b = c // 4, head group g = c % 4 (heads [4g, 4g+4)).
Each core computes q/k/v projections for its 4 heads (columns
[512g, 512g+512) of Wq/Wk/Wv), causal attention with RoPE, and the
partial o_proj contribution  attn_out_local @ Wo[512g:512g+512, :].
The host gather sums the 4 partials per batch (row-parallel linear).

All matmuls run as float32r (full-rate fp32 with ~13-bit multiply
mantissa, measured rel-err ~1.5e-4 per matmul).

Per-core kernel layout (S=2048, D=128, 4 local heads):
  phase 1a: qT,kT [d, s] = Wq/Wk chunk.T @ hsT columns, RoPE fused into
            the PSUM eviction (rotate-half via sbuf->sbuf DMA).
  phase 1b: v [s, d*4h] = hsT chunk.T @ Wv.
  phase 2:  per 512-wide query block j, per head: scoresT [sk, sq]
            blocks (lhsT = kT slice, rhs = qT block), exp on ACT in
            [128, 1024] pairs, causal masking by 0/1-mask multiply,
            AV (lhsT = v block, rhs = expT) and row-sums (lhsT = ones)
            accumulated in PSUM over kb; normalize by 1/L via a K=1
            broadcast matmul; o_proj partial = outT_norm.T @ Wo rows.
"""

import sys

for _p in ("/opt/trn_rl_repo", "/root/.axon_site/_ro/trn_rl_repo"):
    if _p not in sys.path:
        sys.path.insert(0, _p)

import numpy as np

import concourse.bass as bass
import concourse.tile as tile
from concourse import bacc, mybir
from concourse.bass_utils import run_bass_kernel_spmd

f32 = mybir.dt.float32
f32r = mybir.dt.float32r
EXP = mybir.ActivationFunctionType.Exp

B = 2
S = 2048
E = 2048
D = 128
HL = 4          # local heads per core
EL = HL * D     # 512, local projection width
NB = S // 512   # 4 query/key 512-blocks
EC = E // 128   # 16 contraction chunks
SCALE = float(1.0 / np.sqrt(D))

_CACHE = {}


def _build():
    from contextlib import ExitStack

    nc = bacc.Bacc("TRN2", target_bir_lowering=False, debug=False, num_devices=8)

    HST = nc.dram_tensor("hsT", [E, S], f32r, kind="ExternalInput")
    WQ = nc.dram_tensor("wq", [E, EL], f32r, kind="ExternalInput")
    WK = nc.dram_tensor("wk", [E, EL], f32r, kind="ExternalInput")
    WV = nc.dram_tensor("wv", [E, EL], f32r, kind="ExternalInput")
    WO = nc.dram_tensor("wo", [EL, E], f32r, kind="ExternalInput")
    COS = nc.dram_tensor("cosT", [D, S], f32, kind="ExternalInput")
    SIN = nc.dram_tensor("sinTs", [D, S], f32, kind="ExternalInput")  # sign-folded
    MSK = nc.dram_tensor("masks", [128, 4, 512], f32r, kind="ExternalInput")
    ONE = nc.dram_tensor("ones", [128, 129], f32r, kind="ExternalInput")
    OUT = nc.dram_tensor("out", [S, E], f32, kind="ExternalOutput")

    with tile.TileContext(nc) as tc, nc.allow_low_precision("fp32r compute by design"):
        with ExitStack() as octx:
            # kernel-lifetime residents (per-partition KB): kT 32, v 32, masks 8
            res = octx.enter_context(tc.tile_pool(name="res", bufs=1))
            kT = [res.tile([128, S], f32r, tag=f"kT{h}", name=f"kT{h}") for h in range(HL)]
            v_sb = res.tile([128, NB * 4, EL], f32r, tag="v")
            masks = res.tile([128, 4, 512], f32r, tag="masks")
            ones_sb = res.tile([128, 129], f32r, tag="ones_sb")
            nc.sync.dma_start(masks[:], MSK[:])
            nc.sync.dma_start(ones_sb[:], ONE[:])
            ones_col = ones_sb[:, 0:1]
            ones_row = ones_sb[0:1, 1:129]
            dram = octx.enter_context(tc.tile_pool(name="dram", bufs=1, space="DRAM"))
            qts = dram.tile([HL, 128, S], f32r, tag="qts")

            def rope_evict(dst, ps, cos_t, sin_t, tp):
                # dst = raw*cosT + rot(raw)*sinT_signed
                raw = tp.tile([128, 512], f32, tag="qkraw")
                nc.scalar.activation(raw[:], ps[:], mybir.ActivationFunctionType.Copy)
                rot = tp.tile([128, 512], f32, tag="qkrot")
                nc.sync.dma_start(rot[0:64, :], raw[64:128, :])
                nc.sync.dma_start(rot[64:128, :], raw[0:64, :])
                t1 = tp.tile([128, 512], f32, tag="ropet1")
                nc.vector.tensor_mul(t1[:], raw[:], cos_t[:])
                nc.vector.tensor_mul(dst, rot[:], sin_t[:])
                nc.vector.tensor_add(dst, dst, t1[:])

            # ---- phase 1: v, qT (spilled to DRAM), kT ----
            with ExitStack() as ctx:
                wkp = ctx.enter_context(tc.tile_pool(name="wk1", bufs=1))
                wk_sb = wkp.tile([128, EC, EL], f32r, tag="wk")
                nc.sync.dma_start(wk_sb[:], WK[:].rearrange("(c p) m -> p c m", p=128))
                hsp = ctx.enter_context(tc.tile_pool(name="hs1", bufs=2))
                csp = ctx.enter_context(tc.tile_pool(name="cs1", bufs=2))
                tmp = ctx.enter_context(tc.tile_pool(name="tmp1", bufs=2))
                wvp = ctx.enter_context(tc.tile_pool(name="wv1", bufs=3))
                wqp = ctx.enter_context(tc.tile_pool(name="wq1", bufs=2))
                qsp = ctx.enter_context(tc.tile_pool(name="qs1", bufs=2))
                pps = ctx.enter_context(tc.tile_pool(name="pps1", bufs=3, space="PSUM"))
                vps = ctx.enter_context(tc.tile_pool(name="vps1", bufs=4, space="PSUM"))
                for j in range(NB):
                    halves = []
                    for half in range(2):
                        t = hsp.tile([128, EC // 2, 512], f32r, tag="hscol")
                        src = HST[half * 1024:(half + 1) * 1024, j * 512:(j + 1) * 512]
                        nc.sync.dma_start(t[:], src.rearrange("(c p) s -> p c s", p=128))
                        halves.append(t)
                    cos_t = csp.tile([128, 512], f32, tag="cos")
                    sin_t = csp.tile([128, 512], f32, tag="sin")
                    nc.sync.dma_start(cos_t[:], COS[:, j * 512:(j + 1) * 512])
                    nc.sync.dma_start(sin_t[:], SIN[:, j * 512:(j + 1) * 512])

                    # v: stream wv e-chunks, 4 s-subtile psums accumulate over e
                    vp = [vps.tile([128, EL], f32, tag="vps", name=f"vp{j}_{i}") for i in range(4)]
                    for e in range(EC):
                        wv_e = wvp.tile([128, 512], f32r, tag="wve")
                        nc.sync.dma_start(wv_e[:], WV[e * 128:(e + 1) * 128, :])
                        for i in range(4):
                            nc.tensor.matmul(
                                vp[i][:],
                                halves[e // 8][:, e % 8, i * 128:(i + 1) * 128],
                                wv_e[:],
                                start=(e == 0),
                                stop=(e == EC - 1),
                            )
                    for i in range(4):
                        nc.vector.tensor_copy(v_sb[:, j * 4 + i, :], vp[i][:])

                    # qT (to DRAM) and kT (resident): wq streamed per (j, h)
                    for h in range(HL):
                        wq_h = wqp.tile([128, EC, 128], f32r, tag="wqh")
                        nc.sync.dma_start(
                            wq_h[:],
                            WQ[:, h * 128:(h + 1) * 128].rearrange(
                                "(c p) m -> p c m", p=128
                            ),
                        )
                        ps = pps.tile([128, 512], f32, tag="qkps")
                        for e in range(EC):
                            nc.tensor.matmul(
                                ps[:],
                                wq_h[:, e, :],
                                halves[e // 8][:, e % 8, :],
                                start=(e == 0),
                                stop=(e == EC - 1),
                            )
                        qt = qsp.tile([128, 512], f32r, tag="qtile")
                        rope_evict(qt[:], ps[:], cos_t, sin_t, tmp)
                        nc.sync.dma_start(qts[h, :, j * 512:(j + 1) * 512], qt[:])

                        ps = pps.tile([128, 512], f32, tag="qkps")
                        for e in range(EC):
                            nc.tensor.matmul(
                                ps[:],
                                wk_sb[:, e, h * 128:(h + 1) * 128],
                                halves[e // 8][:, e % 8, :],
                                start=(e == 0),
                                stop=(e == EC - 1),
                            )
                        rope_evict(
                            kT[h][:, j * 512:(j + 1) * 512],
                            ps[:], cos_t, sin_t, tmp,
                        )

            # ---- phase 2: attention + o_proj ----
            with ExitStack() as ctx:
                wpool = ctx.enter_context(tc.tile_pool(name="w2", bufs=1))
                wo_sb = wpool.tile([128, HL, E], f32r, tag="wo")
                nc.sync.dma_start(wo_sb[:], WO[:].rearrange("(c p) m -> p c m", p=128))
                qlp = ctx.enter_context(tc.tile_pool(name="ql2", bufs=2))
                sbp = ctx.enter_context(tc.tile_pool(name="sb2", bufs=2))
                onp = ctx.enter_context(tc.tile_pool(name="on2", bufs=5))
                scp = ctx.enter_context(tc.tile_pool(name="scps", bufs=2, space="PSUM"))
                avp = ctx.enter_context(tc.tile_pool(name="avps", bufs=1, space="PSUM"))
                lp = ctx.enter_context(tc.tile_pool(name="lps", bufs=1, space="PSUM"))
                bcp = ctx.enter_context(tc.tile_pool(name="bcps", bufs=1, space="PSUM"))
                opp = ctx.enter_context(tc.tile_pool(name="opps", bufs=1, space="PSUM"))

                for j in range(NB):
                    o_norm = []
                    for h in range(HL):
                        qt = qlp.tile([128, 512], f32r, tag="qld")
                        nc.sync.dma_start(qt[:], qts[h, :, j * 512:(j + 1) * 512])
                        nkb = 4 * j + 4
                        av_ps = avp.tile([128, 512], f32, tag="av")
                        l_ps = lp.tile([1, 512], f32, tag="l")
                        for p in range(nkb // 2):
                            sc_ps = scp.tile([128, 1024], f32, tag="sc")
                            for kk in range(2):
                                kb = 2 * p + kk
                                nc.tensor.matmul(
                                    sc_ps[:, kk * 512:(kk + 1) * 512],
                                    kT[h][:, kb * 128:(kb + 1) * 128],
                                    qt[:],
                                    start=True,
                                    stop=True,
                                )
                            ex = sbp.tile([128, 1024], f32r, tag="expT")
                            nc.scalar.activation(ex[:], sc_ps[:], EXP, scale=SCALE)
                            for kk in range(2):
                                kb = 2 * p + kk
                                m = kb - 4 * j
                                half = ex[:, kk * 512:(kk + 1) * 512]
                                if m >= 0:  # diagonal block: causal mask
                                    nc.vector.tensor_mul(half, half, masks[:, m, :])
                                nc.tensor.matmul(
                                    av_ps[:],
                                    v_sb[:, kb, h * 128:(h + 1) * 128],
                                    half,
                                    start=(kb == 0),
                                    stop=(kb == nkb - 1),
                                )
                                nc.tensor.matmul(
                                    l_ps[:],
                                    ones_col,
                                    half,
                                    start=(kb == 0),
                                    stop=(kb == nkb - 1),
                                )
                        recip = onp.tile([1, 512], f32r, tag="recip")
                        nc.vector.reciprocal(recip[:], l_ps[:])
                        bc_ps = bcp.tile([128, 512], f32, tag="bc")
                        nc.tensor.matmul(
                            bc_ps[:], ones_row, recip[:], start=True, stop=True
                        )
                        bc_sb = onp.tile([128, 512], f32, tag="bcsb")
                        nc.vector.tensor_copy(bc_sb[:], bc_ps[:])
                        on = onp.tile([128, 512], f32r, tag="onorm")
                        nc.vector.tensor_mul(on[:], av_ps[:], bc_sb[:])
                        o_norm.append(on)

                    for i in range(4):
                        orow = sbp.tile([128, E], f32, tag="orow")
                        for n in range(4):
                            op_ps = opp.tile([128, 512], f32, tag="op")
                            for h in range(HL):
                                nc.tensor.matmul(
                                    op_ps[:],
                                    o_norm[h][:, i * 128:(i + 1) * 128],
                                    wo_sb[:, h, n * 512:(n + 1) * 512],
                                    start=(h == 0),
                                    stop=(h == HL - 1),
                                )
                            nc.vector.tensor_copy(
                                orow[:, n * 512:(n + 1) * 512], op_ps[:]
                            )
                        nc.sync.dma_start(
                            OUT[j * 512 + i * 128:j * 512 + (i + 1) * 128, :],
                            orow[:],
                        )

    nc.compile()
    return nc


def _get_nc():
    if "nc" not in _CACHE:
        _CACHE["nc"] = _build()
    return _CACHE["nc"]


def _make_masks():
    sk = np.arange(128)[:, None]
    sq = np.arange(512)[None, :]
    m = np.stack([(sq >= sk + 128 * mm) for mm in range(4)], axis=1)
    return m.astype(np.float32)


def kernel(hidden_states, cos, sin, Wq, Wk, Wv, Wo):
    hidden_states = np.asarray(hidden_states, dtype=np.float32)
    cos = np.asarray(cos, dtype=np.float32)
    sin = np.asarray(sin, dtype=np.float32)
    Wq = np.asarray(Wq, dtype=np.float32)
    Wk = np.asarray(Wk, dtype=np.float32)
    Wv = np.asarray(Wv, dtype=np.float32)
    Wo = np.asarray(Wo, dtype=np.float32)

    nc = _get_nc()
    masks = _make_masks()
    ones_arr = np.ones((128, 129), dtype=np.float32)
    in_maps = []
    hsT = [np.ascontiguousarray(hidden_states[b].T) for b in range(B)]
    cosT = [np.ascontiguousarray(cos[b].T) for b in range(B)]
    sinTs = []
    for b in range(B):
        s = np.ascontiguousarray(sin[b].T)
        s[:64] *= -1.0
        sinTs.append(s)
    for c in range(8):
        b, g = c // 4, c % 4
        cols = slice(512 * g, 512 * (g + 1))
        in_maps.append({
            "hsT": hsT[b],
            "wq": np.ascontiguousarray(Wq[:, cols]),
            "wk": np.ascontiguousarray(Wk[:, cols]),
            "wv": np.ascontiguousarray(Wv[:, cols]),
            "wo": np.ascontiguousarray(Wo[cols, :]),
            "cosT": cosT[b],
            "sinTs": sinTs[b],
            "masks": masks,
            "ones": ones_arr,
        })

    import os
    res = run_bass_kernel_spmd(
        nc, in_maps, core_ids=list(range(8)),
        tmpdir=os.environ.get("BASS_KERNEL_TMPDIR"),
    )
    globals()["LAST_RESULTS"] = res
    globals()["LAST_EXEC_NS"] = res.exec_time_ns
    out = np.empty((B, S, E), dtype=np.float32)
    for b in range(B):
        acc = res.results[4 * b]["out"].astype(np.float32)
        for g in range(1, 4):
            acc = acc + res.results[4 * b + g]["out"]
        out[b] = acc
    return out



# revision 4
# speedup vs baseline: 1.2145x; 1.2145x over previous
"""AttentionWithRoPE on 8 trn2 NeuronCores.

Sharding (tensor-parallel over heads x data-parallel over batch):
  core c -> batch b = c // 4, head group g = c % 4 (heads [4g, 4g+4)).
Each core computes q/k/v projections for its 4 heads (columns
[512g, 512g+512) of Wq/Wk/Wv), causal attention with RoPE, and the
partial o_proj contribution  attn_out_local @ Wo[512g:512g+512, :].
The host gather sums the 4 partials per batch (row-parallel linear).

v2: all matmuls run in bf16 (1 cycle/row, half the DMA/SBUF of fp32),
weights + qT/kT/v fully SBUF-resident (no DRAM spill, no re-loads),
projections/attention/o_proj interleaved per 512-query block j so the
PE stays warm, softmax row-sums accumulated on VectorE + reduced with
a GpSimd partition_all_reduce (drops 176 TensorE matmuls), PSUM tags
packed to exactly 8 banks.

Per-core layout (S=2048, D=128, 4 local heads), per query block j:
  v(j):   4 chains of 16 accumulating matmuls -> v_sb bf16
  per h:  qT/kT chains (16 matmuls), RoPE fused into PSUM eviction
          (rotate-half via sbuf->sbuf DMA, sign folded into sinT)
  attn:   per 1024-wide kb pair: scoresT pair -> exp (ACT, bf16 out)
          -> causal mask mul (diag pairs) -> DVE acc (softmax denom)
          -> AV accumulate; then partition_all_reduce + reciprocal
          + o_norm scale
  oproj:  [128,512] psum groups accumulated over h, DVE evict, DMA out
"""

import os
import sys

for _p in ("/opt/trn_rl_repo", "/root/.axon_site/_ro/trn_rl_repo"):
    if _p not in sys.path:
        sys.path.insert(0, _p)

import numpy as np
import ml_dtypes

import concourse.bass as bass
import concourse.tile as tile
from concourse import bacc, bass_isa, mybir
from concourse.bass_utils import run_bass_kernel_spmd

f32 = mybir.dt.float32
bf16 = mybir.dt.bfloat16
EXP = mybir.ActivationFunctionType.Exp
COPY = mybir.ActivationFunctionType.Copy

B = 2
S = 2048
E = 2048
D = 128
HL = 4          # local heads per core
EL = HL * D     # 512, local projection width
NB = S // 512   # 4 query 512-blocks
EC = E // 128   # 16 contraction chunks
SCALE = float(1.0 / np.sqrt(D))

_CACHE = {}


def _build():
    from contextlib import ExitStack

    nc = bacc.Bacc("TRN2", target_bir_lowering=False, debug=False, num_devices=8)

    HST = nc.dram_tensor("hsT", [E, S], bf16, kind="ExternalInput")
    WQ = nc.dram_tensor("wq", [E, EL], bf16, kind="ExternalInput")
    WK = nc.dram_tensor("wk", [E, EL], bf16, kind="ExternalInput")
    WV = nc.dram_tensor("wv", [E, EL], bf16, kind="ExternalInput")
    WO = nc.dram_tensor("wo", [EL, E], bf16, kind="ExternalInput")
    COS = nc.dram_tensor("cosT", [D, S], f32, kind="ExternalInput")
    SIN = nc.dram_tensor("sinTs", [D, S], f32, kind="ExternalInput")  # sign-folded
    MSK = nc.dram_tensor("masks", [128, 4, 512], bf16, kind="ExternalInput")
    OUT = nc.dram_tensor("out", [S, E], f32, kind="ExternalOutput")

    with tile.TileContext(nc) as tc, nc.allow_low_precision("bf16 compute by design"):
        with ExitStack() as ctx:
            res = ctx.enter_context(tc.tile_pool(name="res", bufs=1))
            wv_sb = res.tile([128, EC, EL], bf16, tag="wv")
            wq_sb = res.tile([128, EC, EL], bf16, tag="wq")
            wk_sb = res.tile([128, EC, EL], bf16, tag="wk")
            wo_sb = res.tile([128, HL, E], bf16, tag="wo")
            cos_sb = res.tile([128, S], f32, tag="cos")
            sin_sb = res.tile([128, S], f32, tag="sin")
            masks = res.tile([128, 4, 512], bf16, tag="masks")
            kT = [res.tile([128, S], bf16, tag=f"kT{h}", name=f"kT{h}") for h in range(HL)]
            qT = [res.tile([128, S], bf16, tag=f"qT{h}", name=f"qT{h}") for h in range(HL)]
            v_sb = res.tile([128, NB * 4, EL], bf16, tag="v")

            # resident loads, chunked so the first matmuls start early
            for c in range(4):
                rows = slice(512 * c, 512 * (c + 1))
                cs = slice(4 * c, 4 * (c + 1))
                nc.sync.dma_start(
                    wv_sb[:, cs, :],
                    WV[rows, :].rearrange("(c p) m -> p c m", p=128),
                )
            nc.sync.dma_start(cos_sb[:], COS[:])
            nc.sync.dma_start(sin_sb[:], SIN[:])
            nc.sync.dma_start(masks[:], MSK[:])
            for c in range(4):
                rows = slice(512 * c, 512 * (c + 1))
                cs = slice(4 * c, 4 * (c + 1))
                nc.sync.dma_start(
                    wq_sb[:, cs, :],
                    WQ[rows, :].rearrange("(c p) m -> p c m", p=128),
                )
                nc.sync.dma_start(
                    wk_sb[:, cs, :],
                    WK[rows, :].rearrange("(c p) m -> p c m", p=128),
                )
            nc.sync.dma_start(wo_sb[:], WO[:].rearrange("(c p) m -> p c m", p=128))

            # working pools
            hsp = ctx.enter_context(tc.tile_pool(name="hsp", bufs=3))
            rawp = ctx.enter_context(tc.tile_pool(name="rawp", bufs=2))
            rotp = ctx.enter_context(tc.tile_pool(name="rotp", bufs=2))
            t1p = ctx.enter_context(tc.tile_pool(name="t1p", bufs=2))
            exp_p = ctx.enter_context(tc.tile_pool(name="exp", bufs=3))
            accp = ctx.enter_context(tc.tile_pool(name="accp", bufs=1))
            acc5p = ctx.enter_context(tc.tile_pool(name="acc5p", bufs=2))
            lrp = ctx.enter_context(tc.tile_pool(name="lrp", bufs=2))
            onp = ctx.enter_context(tc.tile_pool(name="onp", bufs=6))
            outp = ctx.enter_context(tc.tile_pool(name="outp", bufs=3))
            # PSUM: vps 1 + qkps 2 + scps 2 + avps 1 + opps 2 = 8 banks
            vps = ctx.enter_context(tc.tile_pool(name="vps", bufs=1, space="PSUM"))
            qkps = ctx.enter_context(tc.tile_pool(name="qkps", bufs=2, space="PSUM"))
            scps = ctx.enter_context(tc.tile_pool(name="scps", bufs=1, space="PSUM"))
            avps = ctx.enter_context(tc.tile_pool(name="avps", bufs=1, space="PSUM"))
            opps = ctx.enter_context(tc.tile_pool(name="opps", bufs=2, space="PSUM"))

            def rope_evict(dst, ps, j):
                # dst = raw*cosT + rot(raw)*sinT_signed   (fp32 math, bf16 out)
                raw = rawp.tile([128, 512], f32, tag="raw")
                nc.scalar.activation(raw[:], ps[:], COPY)
                rot = rotp.tile([128, 512], f32, tag="rot")
                nc.sync.dma_start(rot[0:64, :], raw[64:128, :])
                nc.sync.dma_start(rot[64:128, :], raw[0:64, :])
                t1 = t1p.tile([128, 512], f32, tag="t1")
                cs = slice(512 * j, 512 * (j + 1))
                nc.vector.tensor_mul(t1[:], raw[:], cos_sb[:, cs])
                nc.vector.tensor_mul(rot[:], rot[:], sin_sb[:, cs])
                nc.vector.tensor_add(dst, t1[:], rot[:])

            for j in range(NB):
                sj = slice(512 * j, 512 * (j + 1))
                halves = []
                for half in range(2):
                    t = hsp.tile([128, EC // 2, 512], bf16, tag="hscol")
                    src = HST[half * 1024:(half + 1) * 1024, sj]
                    nc.sync.dma_start(t[:], src.rearrange("(c p) s -> p c s", p=128))
                    halves.append(t)

                # ---- v projection: 4 sequential 16-matmul chains ----
                for i in range(4):
                    vp = vps.tile([128, EL], f32, tag="vps")
                    for e in range(EC):
                        nc.tensor.matmul(
                            vp[:],
                            halves[e // 8][:, e % 8, i * 128:(i + 1) * 128],
                            wv_sb[:, e, :],
                            start=(e == 0),
                            stop=(e == EC - 1),
                        )
                    nc.vector.tensor_copy(v_sb[:, j * 4 + i, :], vp[:])

                o_norm = []
                for h in range(HL):
                    # ---- q & k projections with fused RoPE eviction ----
                    hs_ = slice(h * 128, (h + 1) * 128)
                    ps = qkps.tile([128, 512], f32, tag="qkps")
                    for e in range(EC):
                        nc.tensor.matmul(
                            ps[:],
                            wq_sb[:, e, hs_],
                            halves[e // 8][:, e % 8, :],
                            start=(e == 0),
                            stop=(e == EC - 1),
                        )
                    rope_evict(qT[h][:, sj], ps, j)

                    ps = qkps.tile([128, 512], f32, tag="qkps")
                    for e in range(EC):
                        nc.tensor.matmul(
                            ps[:],
                            wk_sb[:, e, hs_],
                            halves[e // 8][:, e % 8, :],
                            start=(e == 0),
                            stop=(e == EC - 1),
                        )
                    rope_evict(kT[h][:, sj], ps, j)

                    # ---- attention for (j, h) ----
                    npair = 2 * j + 2
                    nkb = 4 * j + 4
                    av = avps.tile([128, 512], f32, tag="av")
                    acc = accp.tile([128, 2, 512], f32, tag="acc")
                    for p in range(npair):
                        sc = scps.tile([128, 2, 512], f32, tag="sc")
                        for kk in range(2):
                            kb = 2 * p + kk
                            nc.tensor.matmul(
                                sc[:, kk, :],
                                kT[h][:, kb * 128:(kb + 1) * 128],
                                qT[h][:, sj],
                                start=True,
                                stop=True,
                            )
                        ex = exp_p.tile([128, 2, 512], bf16, tag="ex")
                        nc.scalar.activation(ex[:], sc[:], EXP, scale=SCALE)
                        if p >= 2 * j:  # diagonal pairs: causal mask
                            m = 2 * p - 4 * j
                            nc.vector.tensor_mul(ex[:], ex[:], masks[:, m:m + 2, :])
                        if p == 0:
                            nc.vector.tensor_copy(acc[:], ex[:])
                        else:
                            nc.vector.tensor_add(acc[:], acc[:], ex[:])
                        for kk in range(2):
                            kb = 2 * p + kk
                            nc.tensor.matmul(
                                av[:],
                                v_sb[:, kb, hs_],
                                ex[:, kk, :],
                                start=(kb == 0),
                                stop=(kb == nkb - 1),
                            )
                    acc5 = acc5p.tile([128, 512], f32, tag="acc5")
                    nc.vector.tensor_add(acc5[:], acc[:, 0, :], acc[:, 1, :])
                    lr = lrp.tile([128, 512], f32, tag="lr")
                    nc.gpsimd.partition_all_reduce(
                        lr[:], acc5[:], channels=128, reduce_op=bass_isa.ReduceOp.add
                    )
                    nc.vector.reciprocal(lr[:], lr[:])
                    on = onp.tile([128, 512], bf16, tag="onorm")
                    nc.vector.tensor_mul(on[:], av[:], lr[:])
                    o_norm.append(on)

                # ---- o_proj partial for query rows of block j ----
                for i in range(4):
                    rows = slice(512 * j + 128 * i, 512 * j + 128 * (i + 1))
                    for n in range(4):
                        op = opps.tile([128, 512], f32, tag="op")
                        for h in range(HL):
                            nc.tensor.matmul(
                                op[:],
                                o_norm[h][:, i * 128:(i + 1) * 128],
                                wo_sb[:, h, n * 512:(n + 1) * 512],
                                start=(h == 0),
                                stop=(h == HL - 1),
                            )
                        ot = outp.tile([128, 512], f32, tag="out")
                        nc.vector.tensor_copy(ot[:], op[:])
                        nc.sync.dma_start(OUT[rows, n * 512:(n + 1) * 512], ot[:])

    nc.compile()
    return nc


def _get_nc():
    if "nc" not in _CACHE:
        _CACHE["nc"] = _build()
    return _CACHE["nc"]


def _make_masks():
    sk = np.arange(128)[:, None]
    sq = np.arange(512)[None, :]
    m = np.stack([(sq >= sk + 128 * mm) for mm in range(4)], axis=1)
    return m.astype(ml_dtypes.bfloat16)


def kernel(hidden_states, cos, sin, Wq, Wk, Wv, Wo):
    bf = ml_dtypes.bfloat16
    hidden_states = np.asarray(hidden_states, dtype=np.float32)
    cos = np.asarray(cos, dtype=np.float32)
    sin = np.asarray(sin, dtype=np.float32)
    Wq_b = np.asarray(Wq, dtype=np.float32).astype(bf)
    Wk_b = np.asarray(Wk, dtype=np.float32).astype(bf)
    Wv_b = np.asarray(Wv, dtype=np.float32).astype(bf)
    Wo_b = np.asarray(Wo, dtype=np.float32).astype(bf)

    nc = _get_nc()
    masks = _make_masks()
    in_maps = []
    hsT = [np.ascontiguousarray(hidden_states[b].T.astype(bf)) for b in range(B)]
    cosT = [np.ascontiguousarray(cos[b].T) for b in range(B)]
    sinTs = []
    for b in range(B):
        s = np.ascontiguousarray(sin[b].T)
        s[:64] *= -1.0
        sinTs.append(s)
    for c in range(8):
        b, g = c // 4, c % 4
        cols = slice(512 * g, 512 * (g + 1))
        in_maps.append({
            "hsT": hsT[b],
            "wq": np.ascontiguousarray(Wq_b[:, cols]),
            "wk": np.ascontiguousarray(Wk_b[:, cols]),
            "wv": np.ascontiguousarray(Wv_b[:, cols]),
            "wo": np.ascontiguousarray(Wo_b[cols, :]),
            "cosT": cosT[b],
            "sinTs": sinTs[b],
            "masks": masks,
        })

    res = run_bass_kernel_spmd(
        nc, in_maps, core_ids=list(range(8)),
        tmpdir=os.environ.get("BASS_KERNEL_TMPDIR"),
    )
    globals()["LAST_RESULTS"] = res
    globals()["LAST_EXEC_NS"] = res.exec_time_ns
    out = np.empty((B, S, E), dtype=np.float32)
    for b in range(B):
        acc = res.results[4 * b]["out"].astype(np.float32)
        for g in range(1, 4):
            acc = acc + res.results[4 * b + g]["out"]
        out[b] = acc
    return out


# revision 5
# speedup vs baseline: 1.2896x; 1.0618x over previous
"""AttentionWithRoPE on 8 trn2 NeuronCores.

Sharding (tensor-parallel over heads x data-parallel over batch):
  core c -> batch b = c // 4, head group g = c % 4 (heads [4g, 4g+4)).
Each core computes q/k/v projections for its 4 heads (columns
[512g, 512g+512) of Wq/Wk/Wv), causal attention with RoPE, and the
partial o_proj contribution  attn_out_local @ Wo[512g:512g+512, :].
The host gather sums the 4 partials per batch (row-parallel linear).

v3: all matmuls bf16 (1 cycle/row, half the DMA/SBUF of fp32), weights
+ qT/kT/v fully SBUF-resident, per-512-query-block pipeline keeps the
PE warm (projection / attention / o_proj of adjacent blocks overlap).
Softmax denominators accumulate as [1,512] ones-matmuls in PSUM (the
all-ones lhsT column is a slice of the causal mask); 1/L is computed
as exp(-ln L) on the Scalar engine (Ln+Exp+Copy live in one activation
table set) and broadcast across partitions on the idle GpSimd engine,
so no VectorE reciprocal (iterative divide, ~3.3us) and no extra PSUM
bank for a broadcast matmul. RoPE runs in bf16 on VectorE (2x mode).

PSUM banks: proj(v+qk) 2, scores pair 2, av 2, lsum 1, o_proj 1 = 8.
"""

import os
import sys

for _p in ("/opt/trn_rl_repo", "/root/.axon_site/_ro/trn_rl_repo"):
    if _p not in sys.path:
        sys.path.insert(0, _p)

import numpy as np
import ml_dtypes

import concourse.bass as bass
import concourse.tile as tile
from concourse import bacc, bass_isa, mybir
from concourse.bass_utils import run_bass_kernel_spmd

f32 = mybir.dt.float32
bf16 = mybir.dt.bfloat16
EXP = mybir.ActivationFunctionType.Exp
LN = mybir.ActivationFunctionType.Ln
COPY = mybir.ActivationFunctionType.Copy

B = 2
S = 2048
E = 2048
D = 128
HL = 4          # local heads per core
EL = HL * D     # 512, local projection width
NB = S // 512   # 4 query 512-blocks
EC = E // 128   # 16 contraction chunks
SCALE = float(1.0 / np.sqrt(D))

_CACHE = {}


def _build():
    from contextlib import ExitStack

    nc = bacc.Bacc("TRN2", target_bir_lowering=False, debug=False, num_devices=8)

    HST = nc.dram_tensor("hsT", [E, S], bf16, kind="ExternalInput")
    WQ = nc.dram_tensor("wq", [E, EL], bf16, kind="ExternalInput")
    WK = nc.dram_tensor("wk", [E, EL], bf16, kind="ExternalInput")
    WV = nc.dram_tensor("wv", [E, EL], bf16, kind="ExternalInput")
    WO = nc.dram_tensor("wo", [EL, E], bf16, kind="ExternalInput")
    COS = nc.dram_tensor("cosT", [D, S], bf16, kind="ExternalInput")
    SIN = nc.dram_tensor("sinTs", [D, S], bf16, kind="ExternalInput")  # sign-folded
    MSK = nc.dram_tensor("masks", [128, 4, 512], bf16, kind="ExternalInput")
    OUT = nc.dram_tensor("out", [S, E], f32, kind="ExternalOutput")

    with tile.TileContext(nc) as tc, nc.allow_low_precision("bf16 compute by design"):
        with ExitStack() as ctx:
            res = ctx.enter_context(tc.tile_pool(name="res", bufs=1))
            wv_sb = res.tile([128, EC, EL], bf16, tag="wv")
            wq_sb = res.tile([128, EC, EL], bf16, tag="wq")
            wk_sb = res.tile([128, EC, EL], bf16, tag="wk")
            wo_sb = res.tile([128, HL, E], bf16, tag="wo")
            cos_sb = res.tile([128, S], bf16, tag="cos")
            sin_sb = res.tile([128, S], bf16, tag="sin")
            masks = res.tile([128, 4, 512], bf16, tag="masks")
            kT = [res.tile([128, S], bf16, tag=f"kT{h}", name=f"kT{h}") for h in range(HL)]
            qT = [res.tile([128, S], bf16, tag=f"qT{h}", name=f"qT{h}") for h in range(HL)]
            v_sb = res.tile([128, NB * 4, EL], bf16, tag="v")
            # masks[:, 0, 511] is 1 for every sk (sq=511 >= sk for all sk<128)
            # and masks[0, 0, :] is 1 for every sq: free all-ones lhsT columns.
            ones_col = masks[:, 0, 511:512]

            hsp = ctx.enter_context(tc.tile_pool(name="hsp", bufs=4))
            rawp = ctx.enter_context(tc.tile_pool(name="rawp", bufs=2))
            rotp = ctx.enter_context(tc.tile_pool(name="rotp", bufs=2))
            t1p = ctx.enter_context(tc.tile_pool(name="t1p", bufs=2))
            exp_p = ctx.enter_context(tc.tile_pool(name="exp", bufs=3))
            lnp = ctx.enter_context(tc.tile_pool(name="lnp", bufs=2))
            rrp = ctx.enter_context(tc.tile_pool(name="rrp", bufs=2))
            rbp = ctx.enter_context(tc.tile_pool(name="rbp", bufs=2))
            onp = ctx.enter_context(tc.tile_pool(name="onp", bufs=8))
            outp = ctx.enter_context(tc.tile_pool(name="outp", bufs=3))
            # PSUM: proj 2 + sc 2 + av 2 + lsum 1 + op 1 = 8 banks
            pjps = ctx.enter_context(tc.tile_pool(name="pjps", bufs=2, space="PSUM"))
            scps = ctx.enter_context(tc.tile_pool(name="scps", bufs=1, space="PSUM"))
            avps = ctx.enter_context(tc.tile_pool(name="avps", bufs=2, space="PSUM"))
            lsps = ctx.enter_context(tc.tile_pool(name="lsps", bufs=1, space="PSUM"))
            opps = ctx.enter_context(tc.tile_pool(name="opps", bufs=1, space="PSUM"))

            def load_halves(j):
                sj = slice(512 * j, 512 * (j + 1))
                out = []
                for half in range(2):
                    t = hsp.tile([128, EC // 2, 512], bf16, tag="hscol")
                    src = HST[half * 1024:(half + 1) * 1024, sj]
                    nc.sync.dma_start(t[:], src.rearrange("(c p) s -> p c s", p=128))
                    out.append(t)
                return out

            # DMA priority order: wv + first hs block feed the first
            # matmul chains; wo is not needed until the first o_proj.
            for c in range(4):
                rows = slice(512 * c, 512 * (c + 1))
                cs = slice(4 * c, 4 * (c + 1))
                nc.sync.dma_start(
                    wv_sb[:, cs, :],
                    WV[rows, :].rearrange("(c p) m -> p c m", p=128),
                )
            halves_next = load_halves(0)
            for c in range(4):
                rows = slice(512 * c, 512 * (c + 1))
                cs = slice(4 * c, 4 * (c + 1))
                nc.sync.dma_start(
                    wq_sb[:, cs, :],
                    WQ[rows, :].rearrange("(c p) m -> p c m", p=128),
                )
                nc.sync.dma_start(
                    wk_sb[:, cs, :],
                    WK[rows, :].rearrange("(c p) m -> p c m", p=128),
                )
            nc.sync.dma_start(cos_sb[:], COS[:])
            nc.sync.dma_start(sin_sb[:], SIN[:])
            nc.sync.dma_start(masks[:], MSK[:])
            nc.sync.dma_start(wo_sb[:], WO[:].rearrange("(c p) m -> p c m", p=128))

            def rope_evict(dst, ps, j):
                # dst = raw*cosT + rot(raw)*sinT_signed   (bf16 math)
                raw = rawp.tile([128, 512], bf16, tag="raw")
                nc.scalar.activation(raw[:], ps[:], COPY)
                rot = rotp.tile([128, 512], bf16, tag="rot")
                nc.sync.dma_start(rot[0:64, :], raw[64:128, :])
                nc.sync.dma_start(rot[64:128, :], raw[0:64, :])
                t1 = t1p.tile([128, 512], bf16, tag="t1")
                cs = slice(512 * j, 512 * (j + 1))
                nc.vector.tensor_mul(t1[:], raw[:], cos_sb[:, cs])
                nc.vector.tensor_mul(rot[:], rot[:], sin_sb[:, cs])
                nc.vector.tensor_add(dst, t1[:], rot[:])

            for j in range(NB):
                sj = slice(512 * j, 512 * (j + 1))
                halves = halves_next
                if j + 1 < NB:
                    halves_next = load_halves(j + 1)

                # ---- v projection: 4 sequential 16-matmul chains ----
                for i in range(4):
                    vp = pjps.tile([128, EL], f32, tag="pj")
                    for e in range(EC):
                        nc.tensor.matmul(
                            vp[:],
                            halves[e // 8][:, e % 8, i * 128:(i + 1) * 128],
                            wv_sb[:, e, :],
                            start=(e == 0),
                            stop=(e == EC - 1),
                        )
                    nc.vector.tensor_copy(v_sb[:, j * 4 + i, :], vp[:])

                o_norm = []
                for h in range(HL):
                    # ---- q & k projections with fused RoPE eviction ----
                    hs_ = slice(h * 128, (h + 1) * 128)
                    ps = pjps.tile([128, 512], f32, tag="pj")
                    for e in range(EC):
                        nc.tensor.matmul(
                            ps[:],
                            wq_sb[:, e, hs_],
                            halves[e // 8][:, e % 8, :],
                            start=(e == 0),
                            stop=(e == EC - 1),
                        )
                    rope_evict(qT[h][:, sj], ps, j)

                    ps = pjps.tile([128, 512], f32, tag="pj")
                    for e in range(EC):
                        nc.tensor.matmul(
                            ps[:],
                            wk_sb[:, e, hs_],
                            halves[e // 8][:, e % 8, :],
                            start=(e == 0),
                            stop=(e == EC - 1),
                        )
                    rope_evict(kT[h][:, sj], ps, j)

                    # ---- attention for (j, h) ----
                    npair = 2 * j + 2
                    nkb = 4 * j + 4
                    av = avps.tile([128, 512], f32, tag="av")
                    lsum = lsps.tile([1, 512], f32, tag="lsum")
                    for p in range(npair):
                        sc = scps.tile([128, 2, 512], f32, tag="sc")
                        for kk in range(2):
                            kb = 2 * p + kk
                            nc.tensor.matmul(
                                sc[:, kk, :],
                                kT[h][:, kb * 128:(kb + 1) * 128],
                                qT[h][:, sj],
                                start=True,
                                stop=True,
                            )
                        ex = exp_p.tile([128, 2, 512], bf16, tag="ex")
                        nc.scalar.activation(ex[:], sc[:], EXP, scale=SCALE)
                        if p >= 2 * j:  # diagonal pairs: causal mask
                            m = 2 * p - 4 * j
                            nc.vector.tensor_mul(ex[:], ex[:], masks[:, m:m + 2, :])
                        for kk in range(2):
                            kb = 2 * p + kk
                            nc.tensor.matmul(
                                av[:],
                                v_sb[:, kb, hs_],
                                ex[:, kk, :],
                                start=(kb == 0),
                                stop=(kb == nkb - 1),
                            )
                            nc.tensor.matmul(
                                lsum[:],
                                ones_col,
                                ex[:, kk, :],
                                start=(kb == 0),
                                stop=(kb == nkb - 1),
                            )
                    # 1/L = exp(-ln(L)) on ACT; partition-broadcast on GpSimd
                    lnl = lnp.tile([1, 512], f32, tag="lnl")
                    nc.scalar.activation(lnl[:], lsum[:], LN)
                    rrow = rrp.tile([1, 512], f32, tag="rrow")
                    nc.scalar.activation(rrow[:], lnl[:], EXP, scale=-1.0)
                    rbc = rbp.tile([128, 512], f32, tag="rbc")
                    nc.gpsimd.partition_broadcast(rbc[:], rrow[:])
                    on = onp.tile([128, 512], bf16, tag="onorm")
                    nc.vector.tensor_mul(on[:], av[:], rbc[:])
                    o_norm.append(on)

                # ---- o_proj partial for query rows of block j ----
                for i in range(4):
                    rows = slice(512 * j + 128 * i, 512 * j + 128 * (i + 1))
                    for n in range(4):
                        op = opps.tile([128, 512], f32, tag="op")
                        for h in range(HL):
                            nc.tensor.matmul(
                                op[:],
                                o_norm[h][:, i * 128:(i + 1) * 128],
                                wo_sb[:, h, n * 512:(n + 1) * 512],
                                start=(h == 0),
                                stop=(h == HL - 1),
                            )
                        ot = outp.tile([128, 512], f32, tag="out")
                        nc.vector.tensor_copy(ot[:], op[:])
                        nc.sync.dma_start(OUT[rows, n * 512:(n + 1) * 512], ot[:])

    nc.compile()
    return nc


def _get_nc():
    if "nc" not in _CACHE:
        _CACHE["nc"] = _build()
    return _CACHE["nc"]


def _make_masks():
    sk = np.arange(128)[:, None]
    sq = np.arange(512)[None, :]
    m = np.stack([(sq >= sk + 128 * mm) for mm in range(4)], axis=1)
    return m.astype(ml_dtypes.bfloat16)


def kernel(hidden_states, cos, sin, Wq, Wk, Wv, Wo):
    bf = ml_dtypes.bfloat16
    hidden_states = np.asarray(hidden_states, dtype=np.float32)
    cos = np.asarray(cos, dtype=np.float32)
    sin = np.asarray(sin, dtype=np.float32)
    Wq_b = np.asarray(Wq, dtype=np.float32).astype(bf)
    Wk_b = np.asarray(Wk, dtype=np.float32).astype(bf)
    Wv_b = np.asarray(Wv, dtype=np.float32).astype(bf)
    Wo_b = np.asarray(Wo, dtype=np.float32).astype(bf)

    nc = _get_nc()
    masks = _make_masks()
    in_maps = []
    hsT = [np.ascontiguousarray(hidden_states[b].T.astype(bf)) for b in range(B)]
    cosT = [np.ascontiguousarray(cos[b].T.astype(bf)) for b in range(B)]
    sinTs = []
    for b in range(B):
        s = np.ascontiguousarray(sin[b].T)
        s[:64] *= -1.0
        sinTs.append(s.astype(bf))
    for c in range(8):
        b, g = c // 4, c % 4
        cols = slice(512 * g, 512 * (g + 1))
        in_maps.append({
            "hsT": hsT[b],
            "wq": np.ascontiguousarray(Wq_b[:, cols]),
            "wk": np.ascontiguousarray(Wk_b[:, cols]),
            "wv": np.ascontiguousarray(Wv_b[:, cols]),
            "wo": np.ascontiguousarray(Wo_b[cols, :]),
            "cosT": cosT[b],
            "sinTs": sinTs[b],
            "masks": masks,
        })

    res = run_bass_kernel_spmd(
        nc, in_maps, core_ids=list(range(8)),
        tmpdir=os.environ.get("BASS_KERNEL_TMPDIR"),
    )
    globals()["LAST_RESULTS"] = res
    globals()["LAST_EXEC_NS"] = res.exec_time_ns
    out = np.empty((B, S, E), dtype=np.float32)
    for b in range(B):
        acc = res.results[4 * b]["out"].astype(np.float32)
        for g in range(1, 4):
            acc = acc + res.results[4 * b + g]["out"]
        out[b] = acc
    return out


# revision 9
# speedup vs baseline: 1.4751x; 1.1439x over previous
"""AttentionWithRoPE on 8 trn2 NeuronCores.

Sharding (tensor-parallel over heads x data-parallel over batch):
  core c -> batch b = c // 4, head group g = c % 4 (heads [4g, 4g+4)).
Each core computes q/k/v projections for its 4 heads (columns
[512g, 512g+512) of Wq/Wk/Wv), causal attention with RoPE, and the
partial o_proj contribution  attn_out_local @ Wo[512g:512g+512, :].
The host gather sums the 4 partials per batch (row-parallel linear).

v4: all matmuls bf16; weights + qT/kT/v SBUF-resident; inputs arrive
host-pre-rearranged so every DMA is a dense [128, N] tile (8-16KB
contiguous per partition, no gather descriptors). Per-512-query-block
pipeline: projections / attention / o_proj of adjacent blocks overlap
on the PE. Softmax denominators accumulate on VectorE in bf16 (pair
adds, 2x mode), partition-reduced by two [1,512] ones-matmuls per
head (the all-ones lhsT column is a slice of the causal mask), then
1/L = exp(-ln L) batched per block on ScalarE (one Ln + one Exp on
[1,2048] keeps activation-table switches to 2 per block) and
partition-broadcast on the idle GpSimd. av is evicted to SBUF right
after its last accumulation so one PSUM bank suffices for it.

PSUM banks: proj(v+qk) 2, scores pair 2, av 1, lsum 1, o_proj 2 = 8.
"""

import os
import sys

for _p in ("/opt/trn_rl_repo", "/root/.axon_site/_ro/trn_rl_repo"):
    if _p not in sys.path:
        sys.path.insert(0, _p)

import numpy as np
import ml_dtypes

import concourse.bass as bass
import concourse.tile as tile
from concourse import bacc, bass_isa, mybir
from concourse.bass_utils import run_bass_kernel_spmd

f32 = mybir.dt.float32
bf16 = mybir.dt.bfloat16
EXP = mybir.ActivationFunctionType.Exp
LN = mybir.ActivationFunctionType.Ln
COPY = mybir.ActivationFunctionType.Copy

B = 2
S = 2048
E = 2048
D = 128
HL = 4          # local heads per core
EL = HL * D     # 512, local projection width
NB = S // 512   # 4 query 512-blocks
EC = E // 128   # 16 contraction chunks
SCALE = float(1.0 / np.sqrt(D))

_CACHE = {}


def _build():
    from contextlib import ExitStack

    nc = bacc.Bacc("TRN2", target_bir_lowering=False, debug=False, num_devices=8)

    # all pre-rearranged on host: partition dim first, contiguous free dims
    HST = nc.dram_tensor("hsT", [128, 2, NB, EC // 2, 512], bf16, kind="ExternalInput")
    WQ = nc.dram_tensor("wq", [128, EC, EL], bf16, kind="ExternalInput")
    WK = nc.dram_tensor("wk", [128, EC, EL], bf16, kind="ExternalInput")
    WV = nc.dram_tensor("wv", [128, EC, EL], bf16, kind="ExternalInput")
    WO = nc.dram_tensor("wo", [128, HL, E], bf16, kind="ExternalInput")
    COS = nc.dram_tensor("cosT", [D, S], bf16, kind="ExternalInput")
    SIN = nc.dram_tensor("sinTs", [D, S], bf16, kind="ExternalInput")  # sign-folded
    MSK = nc.dram_tensor("masks", [128, 4, 512], bf16, kind="ExternalInput")
    OUT = nc.dram_tensor("out", [S, E], f32, kind="ExternalOutput")

    with tile.TileContext(nc) as tc, nc.allow_low_precision("bf16 compute by design"):
        with ExitStack() as ctx:
            res = ctx.enter_context(tc.tile_pool(name="res", bufs=1))
            wv_sb = res.tile([128, EC, EL], bf16, tag="wv")
            wq_sb = res.tile([128, EC, EL], bf16, tag="wq")
            wk_sb = res.tile([128, EC, EL], bf16, tag="wk")
            wo_sb = res.tile([128, HL, E], bf16, tag="wo")
            cos_sb = res.tile([128, S], bf16, tag="cos")
            sin_sb = res.tile([128, S], bf16, tag="sin")
            masks = res.tile([128, 4, 512], bf16, tag="masks")
            kT = [res.tile([128, S], bf16, tag=f"kT{h}", name=f"kT{h}") for h in range(HL)]
            qT = [res.tile([128, S], bf16, tag=f"qT{h}", name=f"qT{h}") for h in range(HL)]
            v_sb = res.tile([128, NB * 4, EL], bf16, tag="v")
            # masks[:, 0, 511] is 1 for every sk (sq=511 >= sk for all sk<128):
            # a free all-ones lhsT column for the denominator matmuls.
            ones_col = masks[:, 0, 511:512]

            hsp = ctx.enter_context(tc.tile_pool(name="hsp", bufs=3))
            rawp = ctx.enter_context(tc.tile_pool(name="rawp", bufs=2))
            rotp = ctx.enter_context(tc.tile_pool(name="rotp", bufs=2))
            t1p = ctx.enter_context(tc.tile_pool(name="t1p", bufs=2))
            exp_p = ctx.enter_context(tc.tile_pool(name="exp", bufs=3))
            accp = ctx.enter_context(tc.tile_pool(name="accp", bufs=2))
            lrowp = ctx.enter_context(tc.tile_pool(name="lrowp", bufs=1))
            rrp = ctx.enter_context(tc.tile_pool(name="rrp", bufs=1))
            rbp = ctx.enter_context(tc.tile_pool(name="rbp", bufs=1))
            avsp = ctx.enter_context(tc.tile_pool(name="avsp", bufs=8))
            outp = ctx.enter_context(tc.tile_pool(name="outp", bufs=3))
            # PSUM: proj 2 + sc 2 + av 1 + lsum 1 + op 2 = 8 banks
            pjps = ctx.enter_context(tc.tile_pool(name="pjps", bufs=2, space="PSUM"))
            scps = ctx.enter_context(tc.tile_pool(name="scps", bufs=1, space="PSUM"))
            avps = ctx.enter_context(tc.tile_pool(name="avps", bufs=1, space="PSUM"))
            lsps = ctx.enter_context(tc.tile_pool(name="lsps", bufs=1, space="PSUM"))
            opps = ctx.enter_context(tc.tile_pool(name="opps", bufs=2, space="PSUM"))

            def load_halves(j):
                out = []
                for half in range(2):
                    t = hsp.tile([128, EC // 2, 512], bf16, tag="hscol")
                    nc.sync.dma_start(t[:], HST[:, half, j, :, :])
                    out.append(t)
                return out

            # DMA priority order: wv + first hs block feed the first
            # matmul chains; wo is not needed until the first o_proj.
            for c in range(4):
                cs = slice(4 * c, 4 * (c + 1))
                nc.sync.dma_start(wv_sb[:, cs, :], WV[:, cs, :])
            halves_next = load_halves(0)
            for c in range(4):
                cs = slice(4 * c, 4 * (c + 1))
                nc.sync.dma_start(wq_sb[:, cs, :], WQ[:, cs, :])
                nc.sync.dma_start(wk_sb[:, cs, :], WK[:, cs, :])
            nc.sync.dma_start(cos_sb[:], COS[:])
            nc.sync.dma_start(sin_sb[:], SIN[:])
            nc.sync.dma_start(masks[:], MSK[:])
            nc.sync.dma_start(wo_sb[:], WO[:])

            def rope_evict(dst, ps, j):
                # dst = raw*cosT + rot(raw)*sinT_signed   (bf16 math)
                raw = rawp.tile([128, 512], bf16, tag="raw")
                nc.scalar.activation(raw[:], ps[:], COPY)
                rot = rotp.tile([128, 512], bf16, tag="rot")
                nc.sync.dma_start(rot[0:64, :], raw[64:128, :])
                nc.sync.dma_start(rot[64:128, :], raw[0:64, :])
                t1 = t1p.tile([128, 512], bf16, tag="t1")
                cs = slice(512 * j, 512 * (j + 1))
                nc.vector.tensor_mul(t1[:], raw[:], cos_sb[:, cs])
                nc.vector.tensor_mul(rot[:], rot[:], sin_sb[:, cs])
                nc.vector.tensor_add(dst, t1[:], rot[:])

            for j in range(NB):
                sj = slice(512 * j, 512 * (j + 1))
                halves = halves_next
                if j + 1 < NB:
                    halves_next = load_halves(j + 1)

                # ---- v projection: 4 sequential 16-matmul chains ----
                for i in range(4):
                    vp = pjps.tile([128, EL], f32, tag="pj")
                    for e in range(EC):
                        nc.tensor.matmul(
                            vp[:],
                            halves[e // 8][:, e % 8, i * 128:(i + 1) * 128],
                            wv_sb[:, e, :],
                            start=(e == 0),
                            stop=(e == EC - 1),
                        )
                    nc.vector.tensor_copy(v_sb[:, j * 4 + i, :], vp[:])

                av_sb = []
                lrow = lrowp.tile([1, HL, 512], f32, tag="lrow")
                for h in range(HL):
                    # ---- q & k projections with fused RoPE eviction ----
                    hs_ = slice(h * 128, (h + 1) * 128)
                    ps = pjps.tile([128, 512], f32, tag="pj")
                    for e in range(EC):
                        nc.tensor.matmul(
                            ps[:],
                            wq_sb[:, e, hs_],
                            halves[e // 8][:, e % 8, :],
                            start=(e == 0),
                            stop=(e == EC - 1),
                        )
                    rope_evict(qT[h][:, sj], ps, j)

                    ps = pjps.tile([128, 512], f32, tag="pj")
                    for e in range(EC):
                        nc.tensor.matmul(
                            ps[:],
                            wk_sb[:, e, hs_],
                            halves[e // 8][:, e % 8, :],
                            start=(e == 0),
                            stop=(e == EC - 1),
                        )
                    rope_evict(kT[h][:, sj], ps, j)

                    # ---- attention for (j, h) ----
                    npair = 2 * j + 2
                    nkb = 4 * j + 4
                    av = avps.tile([128, 512], f32, tag="av")
                    acc = accp.tile([128, 2, 512], bf16, tag="acc")
                    for p in range(npair):
                        sc = scps.tile([128, 2, 512], f32, tag="sc")
                        for kk in range(2):
                            kb = 2 * p + kk
                            nc.tensor.matmul(
                                sc[:, kk, :],
                                kT[h][:, kb * 128:(kb + 1) * 128],
                                qT[h][:, sj],
                                start=True,
                                stop=True,
                            )
                        ex = exp_p.tile([128, 2, 512], bf16, tag="ex")
                        nc.scalar.activation(ex[:], sc[:], EXP, scale=SCALE)
                        if p >= 2 * j:  # diagonal pairs: causal mask
                            m = 2 * p - 4 * j
                            nc.vector.tensor_mul(ex[:], ex[:], masks[:, m:m + 2, :])
                        if p == 0:
                            nc.vector.tensor_copy(acc[:], ex[:])
                        else:
                            nc.vector.tensor_add(acc[:], acc[:], ex[:])
                        for kk in range(2):
                            kb = 2 * p + kk
                            nc.tensor.matmul(
                                av[:],
                                v_sb[:, kb, hs_],
                                ex[:, kk, :],
                                start=(kb == 0),
                                stop=(kb == nkb - 1),
                            )
                    # partition-reduce the bf16 denominator accumulator
                    lsum = lsps.tile([1, 512], f32, tag="lsum")
                    for kk in range(2):
                        nc.tensor.matmul(
                            lsum[:], ones_col, acc[:, kk, :],
                            start=(kk == 0), stop=(kk == 1),
                        )
                    nc.scalar.activation(lrow[:, h, :], lsum[:], COPY)
                    avs = avsp.tile([128, 512], bf16, tag="avsb")
                    nc.vector.tensor_copy(avs[:], av[:])
                    av_sb.append(avs)

                # 1/L = exp(-ln L), batched over the block's 4 heads,
                # then partition-broadcast on GpSimd.
                lflat = lrow[:].rearrange("p h s -> p (h s)")
                nc.scalar.activation(lflat, lflat, LN)
                rrow = rrp.tile([1, HL * 512], f32, tag="rrow")
                nc.scalar.activation(rrow[:], lflat, EXP, scale=-1.0)
                rbc = rbp.tile([128, HL, 512], f32, tag="rbc")
                nc.gpsimd.partition_broadcast(rbc[:], rrow[:])
                for h in range(HL):
                    nc.vector.tensor_mul(av_sb[h][:], av_sb[h][:], rbc[:, h, :])

                # ---- o_proj partial for query rows of block j ----
                for i in range(4):
                    rows = slice(512 * j + 128 * i, 512 * j + 128 * (i + 1))
                    for n in range(4):
                        op = opps.tile([128, 512], f32, tag="op")
                        for h in range(HL):
                            nc.tensor.matmul(
                                op[:],
                                av_sb[h][:, i * 128:(i + 1) * 128],
                                wo_sb[:, h, n * 512:(n + 1) * 512],
                                start=(h == 0),
                                stop=(h == HL - 1),
                            )
                        ot = outp.tile([128, 512], f32, tag="out")
                        nc.vector.tensor_copy(ot[:], op[:])
                        nc.sync.dma_start(OUT[rows, n * 512:(n + 1) * 512], ot[:])

    nc.compile()
    return nc


def _get_nc():
    if "nc" not in _CACHE:
        _CACHE["nc"] = _build()
    return _CACHE["nc"]


def _make_masks():
    sk = np.arange(128)[:, None]
    sq = np.arange(512)[None, :]
    m = np.stack([(sq >= sk + 128 * mm) for mm in range(4)], axis=1)
    return m.astype(ml_dtypes.bfloat16)


def kernel(hidden_states, cos, sin, Wq, Wk, Wv, Wo):
    bf = ml_dtypes.bfloat16
    hidden_states = np.asarray(hidden_states, dtype=np.float32)
    cos = np.asarray(cos, dtype=np.float32)
    sin = np.asarray(sin, dtype=np.float32)
    Wq_b = np.asarray(Wq, dtype=np.float32).astype(bf)
    Wk_b = np.asarray(Wk, dtype=np.float32).astype(bf)
    Wv_b = np.asarray(Wv, dtype=np.float32).astype(bf)
    Wo_b = np.asarray(Wo, dtype=np.float32).astype(bf)

    nc = _get_nc()
    masks = _make_masks()

    def arrange_w(wcols):  # [E, 512] -> [128, 16, 512], row e = 128c + p
        return np.ascontiguousarray(wcols.reshape(EC, 128, EL).transpose(1, 0, 2))

    def arrange_wo(wrows):  # [512, E] -> [128, 4, E], row = 128h + d
        return np.ascontiguousarray(wrows.reshape(HL, 128, E).transpose(1, 0, 2))

    def arrange_hst(hs_b):  # [S, E] -> hsT [p, half, j, c, s]
        t = hs_b.T.astype(bf)  # [E, S]
        t = t.reshape(2, 8, 128, NB, 512)  # [half, c, p, j, s]
        return np.ascontiguousarray(t.transpose(2, 0, 3, 1, 4))

    in_maps = []
    hsT = [arrange_hst(hidden_states[b]) for b in range(B)]
    cosT = [np.ascontiguousarray(cos[b].T.astype(bf)) for b in range(B)]
    sinTs = []
    for b in range(B):
        s = np.ascontiguousarray(sin[b].T)
        s[:64] *= -1.0
        sinTs.append(s.astype(bf))
    for c in range(8):
        b, g = c // 4, c % 4
        cols = slice(512 * g, 512 * (g + 1))
        in_maps.append({
            "hsT": hsT[b],
            "wq": arrange_w(Wq_b[:, cols]),
            "wk": arrange_w(Wk_b[:, cols]),
            "wv": arrange_w(Wv_b[:, cols]),
            "wo": arrange_wo(Wo_b[cols, :]),
            "cosT": cosT[b],
            "sinTs": sinTs[b],
            "masks": masks,
        })

    res = run_bass_kernel_spmd(
        nc, in_maps, core_ids=list(range(8)),
        tmpdir=os.environ.get("BASS_KERNEL_TMPDIR"),
    )
    globals()["LAST_RESULTS"] = res
    globals()["LAST_EXEC_NS"] = res.exec_time_ns
    out = np.empty((B, S, E), dtype=np.float32)
    for b in range(B):
        acc = res.results[4 * b]["out"].astype(np.float32)
        for g in range(1, 4):
            acc = acc + res.results[4 * b + g]["out"]
        out[b] = acc
    return out


# revision 12
# speedup vs baseline: 1.5526x; 1.0525x over previous
"""AttentionWithRoPE on 8 trn2 NeuronCores.

Sharding (tensor-parallel over heads x data-parallel over batch):
  core c -> batch b = c // 4, head group g = c % 4 (heads [4g, 4g+4)).
Each core computes q/k/v projections for its 4 heads (columns
[512g, 512g+512) of Wq/Wk/Wv), causal attention with RoPE, and the
partial o_proj contribution  attn_out_local @ Wo[512g:512g+512, :].
The host gather sums the 4 partials per batch (row-parallel linear).

v4: all matmuls bf16; weights + qT/kT/v SBUF-resident; inputs arrive
host-pre-rearranged so every DMA is a dense [128, N] tile (8-16KB
contiguous per partition, no gather descriptors). Per-512-query-block
pipeline: projections / attention / o_proj of adjacent blocks overlap
on the PE. Softmax denominators accumulate on VectorE in bf16 (pair
adds, 2x mode), partition-reduced by two [1,512] ones-matmuls per
head (the all-ones lhsT column is a slice of the causal mask), then
1/L = exp(-ln L) batched per block on ScalarE (one Ln + one Exp on
[1,2048] keeps activation-table switches to 2 per block) and
partition-broadcast on the idle GpSimd. av is evicted to SBUF right
after its last accumulation so one PSUM bank suffices for it.

PSUM banks: proj(v+qk) 2, scores pair 2, av 1, lsum 1, o_proj 2 = 8.
"""

import os
import sys

for _p in ("/opt/trn_rl_repo", "/root/.axon_site/_ro/trn_rl_repo"):
    if _p not in sys.path:
        sys.path.insert(0, _p)

import numpy as np
import ml_dtypes

import concourse.bass as bass
import concourse.tile as tile
from concourse import bacc, bass_isa, mybir
from concourse.bass_utils import run_bass_kernel_spmd

f32 = mybir.dt.float32
bf16 = mybir.dt.bfloat16
EXP = mybir.ActivationFunctionType.Exp
LN = mybir.ActivationFunctionType.Ln
COPY = mybir.ActivationFunctionType.Copy

B = 2
S = 2048
E = 2048
D = 128
HL = 4          # local heads per core
EL = HL * D     # 512, local projection width
NB = S // 512   # 4 query 512-blocks
EC = E // 128   # 16 contraction chunks
SCALE = float(1.0 / np.sqrt(D))

_CACHE = {}


class _PinnedActBacc(bacc.Bacc):
    """Pin every activation to the natural_log_exp_and_others table set.

    The stock table-load pass picks, per ACTIVATE, the first act_info set
    containing its function: Exp resolves to exp_and_others and Ln to
    natural_log_exp_and_others, so a kernel using both thrashes table
    loads (~2.7us each). All functions used here (Exp, Ln, Copy) live in
    natural_log_exp_and_others, so blank out every other set's function
    list (indices must keep act_info.json order) and one load suffices.
    """

    def insert_act_table_loads(self):
        from concourse.hw_specs import get_activation_tables

        keep = "natural_log_exp_and_others"
        tables = [
            (n, fns if n == keep else set())
            for n, fns in get_activation_tables(self.m.arch).items()
        ]
        bacc._bass_rust.insert_act_table_loads(self, tables)


def _build():
    from contextlib import ExitStack

    nc = _PinnedActBacc("TRN2", target_bir_lowering=False, debug=False, num_devices=8)

    # all pre-rearranged on host: partition dim first, contiguous free dims
    HST = nc.dram_tensor("hsT", [128, 2, NB, EC // 2, 512], bf16, kind="ExternalInput")
    WQ = nc.dram_tensor("wq", [128, EC, EL], bf16, kind="ExternalInput")
    WK = nc.dram_tensor("wk", [128, EC, EL], bf16, kind="ExternalInput")
    WV = nc.dram_tensor("wv", [128, EC, EL], bf16, kind="ExternalInput")
    WO = nc.dram_tensor("wo", [128, HL, E], bf16, kind="ExternalInput")
    COS = nc.dram_tensor("cosT", [D, S], bf16, kind="ExternalInput")
    SIN = nc.dram_tensor("sinTs", [D, S], bf16, kind="ExternalInput")  # sign-folded
    MSK = nc.dram_tensor("masks", [128, 4, 512], bf16, kind="ExternalInput")
    OUT = nc.dram_tensor("out", [S, E], f32, kind="ExternalOutput")

    with tile.TileContext(nc) as tc, nc.allow_low_precision("bf16 compute by design"):
        with ExitStack() as ctx:
            res = ctx.enter_context(tc.tile_pool(name="res", bufs=1))
            wv_sb = res.tile([128, EC, EL], bf16, tag="wv")
            wq_sb = res.tile([128, EC, EL], bf16, tag="wq")
            wk_sb = res.tile([128, EC, EL], bf16, tag="wk")
            wo_sb = res.tile([128, HL, E], bf16, tag="wo")
            cos_sb = res.tile([128, S], bf16, tag="cos")
            sin_sb = res.tile([128, S], bf16, tag="sin")
            masks = res.tile([128, 4, 512], bf16, tag="masks")
            kT = [res.tile([128, S], bf16, tag=f"kT{h}", name=f"kT{h}") for h in range(HL)]
            qT = [res.tile([128, S], bf16, tag=f"qT{h}", name=f"qT{h}") for h in range(HL)]
            v_sb = res.tile([128, NB * 4, EL], bf16, tag="v")
            # masks[:, 0, 511] is 1 for every sk (sq=511 >= sk for all sk<128):
            # a free all-ones lhsT column for the denominator matmuls.
            ones_col = masks[:, 0, 511:512]

            hsp = ctx.enter_context(tc.tile_pool(name="hsp", bufs=3))
            rawp = ctx.enter_context(tc.tile_pool(name="rawp", bufs=2))
            rotp = ctx.enter_context(tc.tile_pool(name="rotp", bufs=2))
            t1p = ctx.enter_context(tc.tile_pool(name="t1p", bufs=2))
            exp_p = ctx.enter_context(tc.tile_pool(name="exp", bufs=3))
            accp = ctx.enter_context(tc.tile_pool(name="accp", bufs=2))
            lrowp = ctx.enter_context(tc.tile_pool(name="lrowp", bufs=1))
            rrp = ctx.enter_context(tc.tile_pool(name="rrp", bufs=1))
            rbp = ctx.enter_context(tc.tile_pool(name="rbp", bufs=1))
            avsp = ctx.enter_context(tc.tile_pool(name="avsp", bufs=8))
            outp = ctx.enter_context(tc.tile_pool(name="outp", bufs=3))
            # PSUM: proj 2 + sc 2 + av 1 + lsum 1 + op 2 = 8 banks
            pjps = ctx.enter_context(tc.tile_pool(name="pjps", bufs=2, space="PSUM"))
            scps = ctx.enter_context(tc.tile_pool(name="scps", bufs=1, space="PSUM"))
            avps = ctx.enter_context(tc.tile_pool(name="avps", bufs=1, space="PSUM"))
            lsps = ctx.enter_context(tc.tile_pool(name="lsps", bufs=1, space="PSUM"))
            opps = ctx.enter_context(tc.tile_pool(name="opps", bufs=2, space="PSUM"))

            def load_halves(j):
                out = []
                for half in range(2):
                    t = hsp.tile([128, EC // 2, 512], bf16, tag="hscol")
                    nc.sync.dma_start(t[:], HST[:, half, j, :, :])
                    out.append(t)
                return out

            # DMA priority order (sync-engine issue is serial, ~1us per
            # dma_start): first v-chain needs wv chunk 0 + hs block 0;
            # wo is not needed until the first o_proj.
            nc.sync.dma_start(wv_sb[:, 0:4, :], WV[:, 0:4, :])
            halves_next = load_halves(0)
            nc.sync.dma_start(wv_sb[:, 4:16, :], WV[:, 4:16, :])
            nc.sync.dma_start(wq_sb[:], WQ[:])
            nc.sync.dma_start(wk_sb[:], WK[:])
            nc.sync.dma_start(cos_sb[:], COS[:])
            nc.sync.dma_start(sin_sb[:], SIN[:])
            nc.sync.dma_start(masks[:], MSK[:])
            nc.sync.dma_start(wo_sb[:], WO[:])

            def rope_evict(dst, ps, j):
                # dst = raw*cosT + rot(raw)*sinT_signed   (bf16 math)
                raw = rawp.tile([128, 512], bf16, tag="raw")
                nc.vector.tensor_copy(raw[:], ps[:])
                rot = rotp.tile([128, 512], bf16, tag="rot")
                nc.sync.dma_start(rot[0:64, :], raw[64:128, :])
                nc.sync.dma_start(rot[64:128, :], raw[0:64, :])
                t1 = t1p.tile([128, 512], bf16, tag="t1")
                cs = slice(512 * j, 512 * (j + 1))
                nc.vector.tensor_mul(t1[:], raw[:], cos_sb[:, cs])
                nc.vector.tensor_mul(rot[:], rot[:], sin_sb[:, cs])
                nc.vector.tensor_add(dst, t1[:], rot[:])

            for j in range(NB):
                sj = slice(512 * j, 512 * (j + 1))
                halves = halves_next
                if j + 1 < NB:
                    halves_next = load_halves(j + 1)

                # ---- v projection: 4 sequential 16-matmul chains ----
                for i in range(4):
                    vp = pjps.tile([128, EL], f32, tag="pj")
                    for e in range(EC):
                        nc.tensor.matmul(
                            vp[:],
                            halves[e // 8][:, e % 8, i * 128:(i + 1) * 128],
                            wv_sb[:, e, :],
                            start=(e == 0),
                            stop=(e == EC - 1),
                        )
                    nc.vector.tensor_copy(v_sb[:, j * 4 + i, :], vp[:])

                av_sb = []
                lrow = lrowp.tile([1, HL, 512], f32, tag="lrow")
                for h in range(HL):
                    # ---- q & k projections with fused RoPE eviction ----
                    hs_ = slice(h * 128, (h + 1) * 128)
                    ps = pjps.tile([128, 512], f32, tag="pj")
                    for e in range(EC):
                        nc.tensor.matmul(
                            ps[:],
                            wq_sb[:, e, hs_],
                            halves[e // 8][:, e % 8, :],
                            start=(e == 0),
                            stop=(e == EC - 1),
                        )
                    rope_evict(qT[h][:, sj], ps, j)

                    ps = pjps.tile([128, 512], f32, tag="pj")
                    for e in range(EC):
                        nc.tensor.matmul(
                            ps[:],
                            wk_sb[:, e, hs_],
                            halves[e // 8][:, e % 8, :],
                            start=(e == 0),
                            stop=(e == EC - 1),
                        )
                    rope_evict(kT[h][:, sj], ps, j)

                    # ---- attention for (j, h) ----
                    npair = 2 * j + 2
                    nkb = 4 * j + 4
                    av = avps.tile([128, 512], f32, tag="av")
                    acc = accp.tile([128, 2, 512], bf16, tag="acc")
                    for p in range(npair):
                        sc = scps.tile([128, 2, 512], f32, tag="sc")
                        for kk in range(2):
                            kb = 2 * p + kk
                            nc.tensor.matmul(
                                sc[:, kk, :],
                                kT[h][:, kb * 128:(kb + 1) * 128],
                                qT[h][:, sj],
                                start=True,
                                stop=True,
                            )
                        ex = exp_p.tile([128, 2, 512], bf16, tag="ex")
                        nc.scalar.activation(ex[:], sc[:], EXP, scale=SCALE)
                        if p >= 2 * j:  # diagonal pairs: causal mask
                            m = 2 * p - 4 * j
                            nc.vector.tensor_mul(ex[:], ex[:], masks[:, m:m + 2, :])
                        if p == 0:
                            nc.vector.tensor_copy(acc[:], ex[:])
                        else:
                            nc.vector.tensor_add(acc[:], acc[:], ex[:])
                        for kk in range(2):
                            kb = 2 * p + kk
                            nc.tensor.matmul(
                                av[:],
                                v_sb[:, kb, hs_],
                                ex[:, kk, :],
                                start=(kb == 0),
                                stop=(kb == nkb - 1),
                            )
                    # partition-reduce the bf16 denominator accumulator
                    lsum = lsps.tile([1, 512], f32, tag="lsum")
                    for kk in range(2):
                        nc.tensor.matmul(
                            lsum[:], ones_col, acc[:, kk, :],
                            start=(kk == 0), stop=(kk == 1),
                        )
                    nc.scalar.activation(lrow[:, h, :], lsum[:], COPY)
                    avs = avsp.tile([128, 512], bf16, tag="avsb")
                    nc.vector.tensor_copy(avs[:], av[:])
                    av_sb.append(avs)

                # 1/L = exp(-ln L), batched over the block's 4 heads,
                # then partition-broadcast on GpSimd.
                lflat = lrow[:].rearrange("p h s -> p (h s)")
                nc.scalar.activation(lflat, lflat, LN)
                rrow = rrp.tile([1, HL * 512], f32, tag="rrow")
                nc.scalar.activation(rrow[:], lflat, EXP, scale=-1.0)
                rbc = rbp.tile([128, HL, 512], f32, tag="rbc")
                nc.gpsimd.partition_broadcast(rbc[:], rrow[:])
                for h in range(HL):
                    nc.vector.tensor_mul(av_sb[h][:], av_sb[h][:], rbc[:, h, :])

                # ---- o_proj partial for query rows of block j ----
                for i in range(4):
                    rows = slice(512 * j + 128 * i, 512 * j + 128 * (i + 1))
                    for n in range(4):
                        op = opps.tile([128, 512], f32, tag="op")
                        for h in range(HL):
                            nc.tensor.matmul(
                                op[:],
                                av_sb[h][:, i * 128:(i + 1) * 128],
                                wo_sb[:, h, n * 512:(n + 1) * 512],
                                start=(h == 0),
                                stop=(h == HL - 1),
                            )
                        ot = outp.tile([128, 512], f32, tag="out")
                        nc.vector.tensor_copy(ot[:], op[:])
                        nc.sync.dma_start(OUT[rows, n * 512:(n + 1) * 512], ot[:])

    nc.compile()
    return nc


def _get_nc():
    if "nc" not in _CACHE:
        _CACHE["nc"] = _build()
    return _CACHE["nc"]


def _make_masks():
    sk = np.arange(128)[:, None]
    sq = np.arange(512)[None, :]
    m = np.stack([(sq >= sk + 128 * mm) for mm in range(4)], axis=1)
    return m.astype(ml_dtypes.bfloat16)


def kernel(hidden_states, cos, sin, Wq, Wk, Wv, Wo):
    bf = ml_dtypes.bfloat16
    hidden_states = np.asarray(hidden_states, dtype=np.float32)
    cos = np.asarray(cos, dtype=np.float32)
    sin = np.asarray(sin, dtype=np.float32)
    Wq_b = np.asarray(Wq, dtype=np.float32).astype(bf)
    Wk_b = np.asarray(Wk, dtype=np.float32).astype(bf)
    Wv_b = np.asarray(Wv, dtype=np.float32).astype(bf)
    Wo_b = np.asarray(Wo, dtype=np.float32).astype(bf)

    nc = _get_nc()
    masks = _make_masks()

    def arrange_w(wcols):  # [E, 512] -> [128, 16, 512], row e = 128c + p
        return np.ascontiguousarray(wcols.reshape(EC, 128, EL).transpose(1, 0, 2))

    def arrange_wo(wrows):  # [512, E] -> [128, 4, E], row = 128h + d
        return np.ascontiguousarray(wrows.reshape(HL, 128, E).transpose(1, 0, 2))

    def arrange_hst(hs_b):  # [S, E] -> hsT [p, half, j, c, s]
        t = hs_b.T.astype(bf)  # [E, S]
        t = t.reshape(2, 8, 128, NB, 512)  # [half, c, p, j, s]
        return np.ascontiguousarray(t.transpose(2, 0, 3, 1, 4))

    in_maps = []
    hsT = [arrange_hst(hidden_states[b]) for b in range(B)]
    cosT = [np.ascontiguousarray(cos[b].T.astype(bf)) for b in range(B)]
    sinTs = []
    for b in range(B):
        s = np.ascontiguousarray(sin[b].T)
        s[:64] *= -1.0
        sinTs.append(s.astype(bf))
    for c in range(8):
        b, g = c // 4, c % 4
        cols = slice(512 * g, 512 * (g + 1))
        in_maps.append({
            "hsT": hsT[b],
            "wq": arrange_w(Wq_b[:, cols]),
            "wk": arrange_w(Wk_b[:, cols]),
            "wv": arrange_w(Wv_b[:, cols]),
            "wo": arrange_wo(Wo_b[cols, :]),
            "cosT": cosT[b],
            "sinTs": sinTs[b],
            "masks": masks,
        })

    res = run_bass_kernel_spmd(
        nc, in_maps, core_ids=list(range(8)),
        tmpdir=os.environ.get("BASS_KERNEL_TMPDIR"),
    )
    globals()["LAST_RESULTS"] = res
    globals()["LAST_EXEC_NS"] = res.exec_time_ns
    out = np.empty((B, S, E), dtype=np.float32)
    for b in range(B):
        acc = res.results[4 * b]["out"].astype(np.float32)
        for g in range(1, 4):
            acc = acc + res.results[4 * b + g]["out"]
        out[b] = acc
    return out
